# revision 1
# baseline (speedup 1.0000x reference)
"""Cross-attention kernel for Trainium2, SPMD over 8 NeuronCores.

Problem: T=4, B=2, NQ=NK=1024, C=512, H=8 heads (D=64).
  q = clip01(BN0(query @ Wq.T)); k = clip01(BN1(key @ Wk.T)); v = clip01(BN2(value @ Wv.T))
  per head: O = softmax(Q K^T / sqrt(D)) V
  out = BN3(concat(O) @ Wo.T)

Sharding: pure data-parallel, one (t, b) pair per core (T*B == 8 == n_cores).

Per-core dataflow (all layouts chosen so no on-chip transposes are needed):
  - host pre-transposes q/k/v to [C, N] (feature-on-partition) and weights to
    W'.T = (W * bn_scale).T [c_in, d_out]; BN scale folded into weights, BN bias
    added via a K=1 ones-row matmul inside each projection's PSUM accumulation.
  - all matmuls run as float32r (fp32 storage, single-pass PE = 4x fp32 rate;
    measured end-to-end rel err ~2e-4 vs the fp32 reference).
  - q/k projections computed in transposed orientation -> qT/kT [C, N] in SBUF;
    BN bias applied per-partition in the DVE clip epilogue.
  - v projection computed in natural orientation -> V [N, C], stored with a ones
    column appended per head ([128, 8*65] tiles) so each head's PV matmul also
    produces the softmax denominator row for free (bias via a K=1 ones-row MM).
  - scores: S^T[k_idx, q] = kT_h.T @ qT_h per head (K=64 contraction); two heads
    run concurrently in the PE array via row packing (base partitions 0 / 64).
  - q,k in [0,1] => scores in [0,8] => exp needs no max-subtraction.
  - E^T = exp(S^T * 0.125) on ScalarE, PV: U^T[d,q] (+denom) = [V_h|1].T @ E^T.
  - attention is processed chunk-major (all key-tiles for q-chunk 0, then
    chunk 1) so softmax normalization + U-PSUM recycling happen per chunk and
    the first chunk only needs half the q/k input DMA.
  - normalize: recip(denom row) broadcast to 64 partitions (gpsimd), multiplied
    on DVE while copying U^T from PSUM into oT [C, N].
  - out projection accumulated as per-head-pair partials into an SBUF
    accumulator (spread across later attention phases), natural [N, C] output.
  - emission interleaves next-phase projections and partials into each
    attention sweep so PE/ACT/DVE/Pool stay co-busy; ACT exp table prefetched.
"""

import numpy as np

import os

H, D, C, N = 8, 64, 512, 1024
PS_BUFS = int(os.environ.get("K_PS", "4"))   # PSUM pools: PS + PJ + 4 U <= 8
PJ_BUFS = int(os.environ.get("K_PJ", "2"))
DMA_ILV = int(os.environ.get("K_DMAILV", "0"))  # interleave q/k chunk-0 halves
U_BUFS = int(os.environ.get("K_U", "2"))
V_FIRST = int(os.environ.get("K_VFIRST", "0"))  # project V before q/k path
CT = C // 128          # 4 c-tiles
NT = N // 128          # 8 n-tiles
CH = N // 512          # 2 free-dim chunks of 512
EPS = 1e-5
N_CORES = 8

_CACHE = {}


def _build():
    from contextlib import ExitStack

    import concourse.bass as bass
    import concourse.tile as tile
    from concourse import bacc, mybir

    f32 = mybir.dt.float32
    f32r = mybir.dt.float32r
    ts = bass.ts

    nc = bacc.Bacc("TRN2", target_bir_lowering=False, debug=False,
                   num_devices=N_CORES)

    xq = nc.dram_tensor("xq", [C, N], f32r, kind="ExternalInput").ap()
    xk = nc.dram_tensor("xk", [C, N], f32r, kind="ExternalInput").ap()
    xv = nc.dram_tensor("xv", [C, N], f32r, kind="ExternalInput").ap()
    wq = nc.dram_tensor("wq", [C, C], f32r, kind="ExternalInput").ap()
    wk = nc.dram_tensor("wk", [C, C], f32r, kind="ExternalInput").ap()
    wv = nc.dram_tensor("wv", [C, C], f32r, kind="ExternalInput").ap()
    wo = nc.dram_tensor("wo", [C, C], f32r, kind="ExternalInput").ap()
    tbias = nc.dram_tensor("tbias", [4, C], f32r, kind="ExternalInput").ap()
    tbt = nc.dram_tensor("tbt", [8, 128], f32, kind="ExternalInput").ap()
    out = nc.dram_tensor("out", [N, C], f32, kind="ExternalOutput").ap()

    Exp = mybir.ActivationFunctionType.Exp
    MAX, MIN = mybir.AluOpType.max, mybir.AluOpType.min

    def mm(out_ap, lhsT, rhs, **kw):
        # operands are float32r tiles: fp32 storage, single-pass PE
        nc.tensor.matmul(out_ap, lhsT=lhsT, rhs=rhs, **kw)

    with tile.TileContext(nc) as tc, ExitStack() as ctx:
        sb = ctx.enter_context(tc.tile_pool(name="sb", bufs=1))
        qk = ctx.enter_context(tc.tile_pool(name="qk", bufs=int(os.environ.get("K_QK", "2"))))
        ep = ctx.enter_context(tc.tile_pool(name="ep", bufs=int(os.environ.get("K_EP", "7"))))
        yp = ctx.enter_context(tc.tile_pool(name="yp", bufs=int(os.environ.get("K_YP", "5"))))
        ps = ctx.enter_context(tc.tile_pool(name="ps", bufs=PS_BUFS, space="PSUM"))
        pj = ctx.enter_context(tc.tile_pool(name="pj", bufs=PJ_BUFS, space="PSUM")) if PJ_BUFS else ps
        up = ctx.enter_context(tc.tile_pool(name="up", bufs=U_BUFS, space="PSUM"))

        ones = sb.tile([1, N], f32r, tag="ones")
        nc.gpsimd.memset(ones[:].bitcast(f32), 1.0)
        # warm the ACT exp table while input DMAs stream
        junk = sb.tile([1, 8], f32, tag="junk")
        nc.scalar.activation(junk[:], ones[0:1, 0:8], Exp)
        # keep the PE array busy through the input-DMA window so the clock
        # (HAM) is fully ramped when the first real matmuls issue
        for w in range(int(os.environ.get("K_WARM", "0"))):
            pw = pj.tile([128, 64], f32, tag="pj" if PJ_BUFS else "ps",
                         name=f"warm{w}")
            mm(pw[:], ones[0:1, 0:128], rhs=ones[0:1, 0:64],
               start=True, stop=True, skip_group_check=True)

        def load_w(name, ap):
            # all 4 c-tiles in one SBUF tensor [128, 4*C], one DMA
            t = sb.tile([128, CT * C], f32r, tag=name, name=name)
            nc.sync.dma_start(t[:].rearrange("p (c d) -> p c d", d=C),
                              ap[:].rearrange("(c p) d -> p c d", p=128))
            return [t[:, ts(ck, C)] for ck in range(CT)]

        def load_x_alloc(name, ap):
            t = sb.tile([128, CT * N], f32r, tag=name, name=name)
            tv = t[:].rearrange("p (c n) -> p c n", n=N)
            av = ap[:].rearrange("(c p) n -> p c n", p=128)
            def half(ch):
                nc.sync.dma_start(tv[:, :, ts(ch, 512)], av[:, :, ts(ch, 512)])
            return [t[:, ts(ck, N)] for ck in range(CT)], half

        # bias rows in one tile [1, 4*C], one DMA (tiny, load first)
        tbs = sb.tile([1, 4 * C], f32r, tag="tbs")
        nc.sync.dma_start(tbs[:].rearrange("p (j c) -> p j c", c=C),
                          tbias[:].rearrange("(j p) c -> p j c", p=1))
        tb = [tbs[0:1, ts(j, C)] for j in range(4)]
        # transposed q/k biases: [128, 8], col j*4+jj = bias of proj j, d-tile jj
        tbt_sb = sb.tile([128, 8], f32, tag="tbt")
        nc.sync.dma_start(tbt_sb[:], tbt[:].rearrange("f p -> p f"))
        if V_FIRST:
            wv_t = load_w("wv", wv)
            xv_t, xv_half = load_x_alloc("xv", xv)
            xv_half(0)
            xv_half(1)
        wq_t = load_w("wq", wq)
        xq_t, xq_half = load_x_alloc("xq", xq)
        xq_half(0)
        if not DMA_ILV:
            xq_half(1)
        wk_t = load_w("wk", wk)
        xk_t, xk_half = load_x_alloc("xk", xk)
        xk_half(0)
        if DMA_ILV:
            xq_half(1)
        xk_half(1)
        if not V_FIRST:
            wv_t = load_w("wv", wv)
            xv_t, xv_half = load_x_alloc("xv", xv)
            xv_half(0)
            xv_half(1)
        wo_t = load_w("wo", wo)

        def proj_t(w_tiles, x_tiles, bias_col, j, key):
            """Transposed-orientation projection d-tile j -> [128, N] clipped."""
            dst = qk.tile([128, N], f32r, tag=f"p{key}", name=f"p{key}{j}")
            for ch in range(CH):
                p = pj.tile([128, 512], f32, tag="pj" if PJ_BUFS else "ps")
                for ck in range(CT):
                    mm(p[:], w_tiles[ck][:, ts(j, 128)],
                       rhs=x_tiles[ck][:, ts(ch, 512)],
                       start=(ck == 0), stop=(ck == CT - 1),
                       skip_group_check=True)
                nc.vector.tensor_scalar(dst[:, ts(ch, 512)], p[:], bias_col,
                                        0.0, mybir.AluOpType.add, MAX)
                nc.gpsimd.tensor_scalar_min(dst[:, ts(ch, 512)],
                                            dst[:, ts(ch, 512)], 1.0)
            return dst

        # V projection (natural orientation), with ones column per head
        V_t = [None] * NT

        def v_proj(m):
            vt = sb.tile([128, H * (D + 1)], f32r, tag=f"V{m}", name=f"V{m}")
            vv = vt[:].rearrange("p (h x) -> p h x", x=D + 1)
            nc.gpsimd.memset(vv[:, :, D:D + 1].bitcast(f32), 1.0)
            p = pj.tile([128, 512], f32, tag="pj" if PJ_BUFS else "ps")
            for ck in range(CT):
                mm(p[:], xv_t[ck][:, ts(m, 128)],
                   rhs=wv_t[ck][:], start=(ck == 0), stop=False,
                   skip_group_check=True)
            mm(p[:], ones[0:1, 0:128], rhs=tb[2],
               start=False, stop=True, skip_group_check=True)
            nc.vector.tensor_scalar(vv[:, :, 0:D],
                                    p[:].rearrange("p (h x) -> p h x", x=D),
                                    0.0, 1.0, MAX, MIN)
            V_t[m] = vt

        # oT: normalized attention output, [C, N] transposed (lhsT of out-proj)
        oT = [sb.tile([128, N], f32r, tag=f"oT{j}", name=f"oT{j}") for j in range(CT)]
        # out-projection partial accumulator [N-tile, dout] per m, in SBUF
        y_acc = sb.tile([128, NT * 512], f32, tag="y_acc")

        def out_proj_partial(hp, m):
            pool = ps if hp == 3 else pj
            p = pool.tile([128, 512], f32, tag="ps" if (hp == 3 or not PJ_BUFS) else "pj")
            mm(p[:], oT[hp][:, ts(m, 128)], rhs=wo_t[hp][:],
               start=True, stop=(hp < 3), skip_group_check=True)
            if hp == 3:
                mm(p[:], ones[0:1, 0:128], rhs=tb[3],
                   start=False, stop=True, skip_group_check=True)
            acc = y_acc[:, ts(m, 512)]
            if hp == 0:
                nc.vector.tensor_copy(acc, p[:])
            elif hp < 3:
                nc.vector.tensor_tensor(acc, p[:], acc, mybir.AluOpType.add)
            else:
                y = yp.tile([128, 512], f32, tag="y")
                nc.vector.tensor_tensor(y[:], p[:], acc, mybir.AluOpType.add)
                nc.sync.dma_start(out[ts(m, 128), :], y[:])

        hoist = {}

        def qk_exp_g(hp, qP, kP, m, ch):
            E = {}
            for h in (2 * hp, 2 * hp + 1):
                base = (h % 2) * 64
                sps = ps.tile([128, 512], f32, tag="ps")
                mm(sps[:], kP[base:base + 64, ts(m, 128)],
                   rhs=qP[base:base + 64, ts(ch, 512)],
                   start=True, stop=True, skip_group_check=True)
                E[h] = ep.tile([128, 512], f32r, tag="E",
                               name=f"E{h}_{m}_{ch}")
                nc.scalar.activation(E[h][:], sps[:], Exp,
                                     scale=float(D) ** -0.5)
            return E

        def attention(hp, qP, kP):
            heads = (2 * hp, 2 * hp + 1)

            def qk_exp(m, ch):
                return qk_exp_g(hp, qP, kP, m, ch)

            for ch in range(CH):
                U = {h: up.tile([D + 1, 512], f32, tag="U", name=f"U{h}_{ch}")
                     for h in heads}
                for m in range(NT):
                    E = hoist.pop((hp, ch, m), None) or qk_exp(m, ch)
                    if hp == 0 and ch == 0 and not V_FIRST:
                        v_proj(m)
                    for h in heads:
                        mm(U[h][:], V_t[m][:, h * (D + 1):(h + 1) * (D + 1)],
                           rhs=E[h][:], start=(m == 0), stop=(m == NT - 1),
                           skip_group_check=True)
                    if hp > 0 and ch == 0:
                        out_proj_partial(hp - 1, m)
                    if hp == 3 and ch == 1 and m < 4:
                        out_proj_partial(3, m)
                    if hp < 3 and m == 3:
                        if ch == 0:
                            nxt["q"] = proj_t(wq_t, xq_t,
                                              tbt_sb[:, hp + 1:hp + 2],
                                              hp + 1, "q")
                        else:
                            nxt["k"] = proj_t(wk_t, xk_t,
                                              tbt_sb[:, 5 + hp:6 + hp],
                                              hp + 1, "k")
                # pre-issue the next sweep's first QK+exp so ACT never drains
                if ch == 0:
                    hoist[(hp, 1, 0)] = qk_exp(0, 1)
                elif hp < 3:
                    hoist[(hp + 1, 0, 0)] = qk_exp_g(hp + 1, nxt["q"],
                                                     nxt["k"], 0, 0)
                # per-chunk softmax normalization (denominator in U row D)
                for h in heads:
                    rc = yp.tile([1, 512], f32, tag="rc")
                    nc.vector.reciprocal(rc[:], U[h][D:D + 1, :])
                    B = yp.tile([64, 512], f32, tag="B")
                    nc.gpsimd.partition_broadcast(B[:], rc[0:1, :], channels=64)
                    base = (h % 2) * 64
                    nc.vector.tensor_mul(oT[hp][base:base + 64, ts(ch, 512)],
                                         U[h][0:D, :], B[:])
                if hp == 3 and ch == 1:
                    for m in range(4, NT):
                        out_proj_partial(3, m)

        if V_FIRST:
            for m in range(NT):
                v_proj(m)
        nxt = {"q": proj_t(wq_t, xq_t, tbt_sb[:, 0:1], 0, "q"),
               "k": proj_t(wk_t, xk_t, tbt_sb[:, 4:5], 0, "k")}
        for hp in range(4):
            attention(hp, nxt["q"], nxt["k"])

    nc.compile()
    return nc


def get_nc():
    if "nc" not in _CACHE:
        _CACHE["nc"] = _build()
    return _CACHE["nc"]


def _prep_inputs(query, key, value, Wq, Wk, Wv, Wo, bn_params):
    """Host-side: shard + transpose + fold BN scale into weights."""
    query = np.ascontiguousarray(np.asarray(query, dtype=np.float32))
    key = np.ascontiguousarray(np.asarray(key, dtype=np.float32))
    value = np.ascontiguousarray(np.asarray(value, dtype=np.float32))
    bn = np.asarray(bn_params, dtype=np.float32)

    s = bn[:, 0] / np.sqrt(bn[:, 3] + EPS)      # [4, C]
    t = bn[:, 1] - bn[:, 2] * s                  # [4, C]

    def wprep(W, j):
        W = np.asarray(W, dtype=np.float32)
        return np.ascontiguousarray((W * s[j][:, None]).T)

    wqT, wkT, wvT, woT = (wprep(Wq, 0), wprep(Wk, 1), wprep(Wv, 2),
                          wprep(Wo, 3))
    tbias = np.ascontiguousarray(t)
    # transposed q/k biases: rows (proj, d-tile) of 128
    tbt = np.ascontiguousarray(
        np.concatenate([t[0].reshape(4, 128), t[1].reshape(4, 128)]))

    # [T, B, N, C] -> [8, C, N]
    def xT(x):
        return np.ascontiguousarray(
            x.reshape(N_CORES, N, C).transpose(0, 2, 1))

    qT, kT, vT = xT(query), xT(key), xT(value)

    in_maps = []
    for i in range(N_CORES):
        in_maps.append({
            "xq": qT[i], "xk": kT[i], "xv": vT[i],
            "wq": wqT, "wk": wkT, "wv": wvT, "wo": woT,
            "tbias": tbias, "tbt": tbt,
        })
    return in_maps


def kernel(query, key, value, Wq, Wk, Wv, Wo, bn_params):
    from concourse.bass_utils import run_bass_kernel_spmd

    nc = get_nc()
    in_maps = _prep_inputs(query, key, value, Wq, Wk, Wv, Wo, bn_params)
    res = run_bass_kernel_spmd(nc, in_maps, core_ids=list(range(N_CORES)),
                               trace=False)
    T, B = 4, 2
    out = np.stack([res.results[i]["out"] for i in range(N_CORES)])
    return np.ascontiguousarray(out.reshape(T, B, N, C).astype(np.float32))



# revision 2
# speedup vs baseline: 1.1290x; 1.1290x over previous
"""Cross-attention kernel for Trainium2, SPMD over 8 NeuronCores.

Problem: T=4, B=2, NQ=NK=1024, C=512, H=8 heads (D=64).
  q = clip01(BN0(query @ Wq.T)); k = clip01(BN1(key @ Wk.T)); v = clip01(BN2(value @ Wv.T))
  per head: O = softmax(Q K^T / sqrt(D)) V
  out = BN3(concat(O) @ Wo.T)

Sharding: pure data-parallel, one (t, b) pair per core (T*B == 8 == n_cores).

v2 dataflow (fp8e4 + DoubleRow attention core; ScalarE-exp is the bottleneck):
  - host pre-transposes q/k/v to [C, N] and weights to W'.T with BN scale
    folded; all BN biases applied via K=1 ones-row matmuls in PSUM.
  - projections run in float32r; q/k epilogues emit fp8e4 tiles (clip01 +
    round), v epilogue emits fp8e4 V-pair tiles.
  - QK per (head, m): DoubleRow matmul with k-slots (m, m+1) as weights and
    q-slots (data, zeros) as the moving operand -> S^T[m] at half PE cost;
    scores land in 2-bank PSUM tiles per m-pair.
  - exp: one ScalarE op per (head, m-pair): E = exp(0.125*S - 2.75) -> fp8
    (max |E| ~190 < fp8e4 max 240; the uniform shift cancels in softmax).
  - PV: genuine DoubleRow over m-pairs: lhsT = [V_h | ones64] pair tile
    [128,2,128], rhs = E pair tile [128,2,512]; U rows 0:64 = numerator,
    rows 64:128 = softmax denominator replicated (ones columns) so
    normalization is recip + mult on DVE with no partition broadcast.
  - out projection accumulates all 4 head-pair tiles in PSUM per output
    m-tile at end of each q-chunk; bias via ones-row matmul; DVE copy + DMA.
  - input DMAs split across the SP and Activation HWDGE queues.
"""

import numpy as np

import os

H, D, C, N = 8, 64, 512, 1024
CT = C // 128          # 4 c-tiles
NT = N // 128          # 8 n-tiles
CH = N // 512          # 2 free-dim chunks of 512
EPS = 1e-5
N_CORES = 8
K_WARM = int(os.environ.get("K_WARM", "0"))
EP_BUFS = int(os.environ.get("K_EP", "3"))
SC_BUFS = int(os.environ.get("K_SC", "2"))

_CACHE = {}


def _build():
    from contextlib import ExitStack

    import concourse.bass as bass
    import concourse.tile as tile
    from concourse import bacc, mybir

    f32 = mybir.dt.float32
    f32r = mybir.dt.float32r
    f8 = mybir.dt.float8e4
    DR = mybir.MatmulPerfMode.DoubleRow
    ts = bass.ts

    nc = bacc.Bacc("TRN2", target_bir_lowering=False, debug=False,
                   num_devices=N_CORES)

    xq = nc.dram_tensor("xq", [C, N], f32r, kind="ExternalInput").ap()
    xk = nc.dram_tensor("xk", [C, N], f32r, kind="ExternalInput").ap()
    xv = nc.dram_tensor("xv", [C, N], f32r, kind="ExternalInput").ap()
    wq = nc.dram_tensor("wq", [C, C], f32r, kind="ExternalInput").ap()
    wk = nc.dram_tensor("wk", [C, C], f32r, kind="ExternalInput").ap()
    wv = nc.dram_tensor("wv", [C, C], f32r, kind="ExternalInput").ap()
    wo = nc.dram_tensor("wo", [C, C], f32r, kind="ExternalInput").ap()
    tbias = nc.dram_tensor("tbias", [4, C], f32r, kind="ExternalInput").ap()
    out = nc.dram_tensor("out", [N, C], f32, kind="ExternalOutput").ap()

    Exp = mybir.ActivationFunctionType.Exp
    MAX, MIN = mybir.AluOpType.max, mybir.AluOpType.min

    def mm(out_ap, lhsT, rhs, **kw):
        nc.tensor.matmul(out_ap, lhsT=lhsT, rhs=rhs, **kw)

    with tile.TileContext(nc) as tc, ExitStack() as ctx:
        sb = ctx.enter_context(tc.tile_pool(name="sb", bufs=1))
        ep = ctx.enter_context(tc.tile_pool(name="ep", bufs=EP_BUFS))
        yp = ctx.enter_context(tc.tile_pool(name="yp", bufs=3))
        sc = ctx.enter_context(tc.tile_pool(name="sc", bufs=SC_BUFS, space="PSUM"))
        up = ctx.enter_context(tc.tile_pool(name="up", bufs=2, space="PSUM"))
        pj = ctx.enter_context(tc.tile_pool(name="pj", bufs=2, space="PSUM"))

        ones = sb.tile([1, 512], f32r, tag="ones")
        nc.gpsimd.memset(ones[:].bitcast(f32), 1.0)
        ebias = sb.tile([128, 1], f32, tag="ebias")
        nc.gpsimd.memset(ebias[:], -2.75)
        # warm the ACT exp table while input DMAs stream
        junk = sb.tile([1, 8], f32, tag="junk")
        nc.scalar.activation(junk[:], ones[0:1, 0:8].bitcast(f32), Exp)
        for w in range(K_WARM):
            pw = pj.tile([128, 512], f32, tag="pj", name=f"warm{w}")
            mm(pw[:], ones[0:1, 0:128], rhs=ones[0:1, 0:512],
               start=True, stop=True, skip_group_check=True)

        # ---- input DMAs, split across SP + ACT hwdge queues ----
        def load_w(name, ap, eng):
            t = sb.tile([128, CT * C], f32r, tag=name, name=name)
            eng.dma_start(t[:].rearrange("p (c d) -> p c d", d=C),
                          ap[:].rearrange("(c p) d -> p c d", p=128))
            return [t[:, ts(ck, C)] for ck in range(CT)]

        def load_x_alloc(name, ap, eng):
            t = sb.tile([128, CT * N], f32r, tag=name, name=name)
            tv = t[:].rearrange("p (c n) -> p c n", n=N)
            av = ap[:].rearrange("(c p) n -> p c n", p=128)
            def half(ch):
                eng.dma_start(tv[:, :, ts(ch, 512)], av[:, :, ts(ch, 512)])
            return [t[:, ts(ck, N)] for ck in range(CT)], half

        # bias rows first (tiny)
        tbs = sb.tile([1, 4 * C], f32r, tag="tbs")
        nc.sync.dma_start(tbs[:].rearrange("p (j c) -> p j c", c=C),
                          tbias[:].rearrange("(j p) c -> p j c", p=1))
        tb = [tbs[0:1, ts(j, C)] for j in range(4)]

        # SP queue: wq, xq; ACT queue: wk, xk; then v/o split across both
        wq_t = load_w("wq", wq, nc.sync)
        wk_t = load_w("wk", wk, nc.scalar)
        xq_t, xq_half = load_x_alloc("xq", xq, nc.sync)
        xk_t, xk_half = load_x_alloc("xk", xk, nc.scalar)
        xq_half(0)
        xk_half(0)
        wv_t = load_w("wv", wv, nc.sync)
        xv_t, xv_half = load_x_alloc("xv", xv, nc.scalar)
        xv_half(0)
        xq_half(1)
        xk_half(1)
        xv_half(1)
        wo_t = load_w("wo", wo, nc.sync)

        # ---- persistent fp8 operand tiles ----
        # qP[hp]: [128, (ch, slot, 512)]; slot1 = zeros (DoubleRow spacer)
        qP = [sb.tile([128, 2048], f8, tag=f"qP{j}", name=f"qP{j}")
              for j in range(CT)]
        # kf[hp]: [128, 1152]; m-major k columns + 128 pad cols
        kf = [sb.tile([128, 1152], f8, tag=f"kf{j}", name=f"kf{j}")
              for j in range(CT)]
        # V pair tiles: [128, (slot, head, 128)]; per head 64 V + 64 ones
        Vt = [sb.tile([128, 2048], f8, tag=f"V{a}", name=f"V{a}")
              for a in range(NT // 2)]
        for j in range(CT):
            z = qP[j][:].rearrange("p (c s n) -> p c s n", s=2, n=512)
            nc.gpsimd.memset(z[:, :, 1, :], 0.0)
            nc.gpsimd.memset(kf[j][:, 1024:1152], 0.0)
        for a in range(NT // 2):
            v4 = Vt[a][:].rearrange("p (s h x) -> p s h x", s=2, x=128)
            nc.gpsimd.memset(v4[:, :, :, 64:128], 1.0)

        # oT: normalized attention output [C, N] (lhsT of out-projection)
        oT = [sb.tile([128, N], f32r, tag=f"oT{j}", name=f"oT{j}")
              for j in range(CT)]

        def q_proj(hp, ch):
            """q d-tile hp, chunk ch -> qP[hp] slot0, clipped fp8."""
            p = pj.tile([128, 512], f32, tag="pj")
            for ck in range(CT):
                mm(p[:], wq_t[ck][:, ts(hp, 128)], rhs=xq_t[ck][:, ts(ch, 512)],
                   start=(ck == 0), stop=False, skip_group_check=True)
            mm(p[:], tbs[0:1, hp * 128 + 0 * C:hp * 128 + 0 * C + 128],
               rhs=ones[0:1, 0:512], start=False, stop=True,
               skip_group_check=True)
            nc.vector.tensor_scalar(qP[hp][:, ch * 1024:ch * 1024 + 512],
                                    p[:], 0.0, 1.0, MAX, MIN)

        def k_proj(hp, c):
            """k d-tile hp, k-position chunk c (m-tiles 4c..4c+3) -> kf."""
            p = pj.tile([128, 512], f32, tag="pj")
            for ck in range(CT):
                mm(p[:], wk_t[ck][:, ts(hp, 128)], rhs=xk_t[ck][:, ts(c, 512)],
                   start=(ck == 0), stop=False, skip_group_check=True)
            mm(p[:], tbs[0:1, C + hp * 128:C + hp * 128 + 128],
               rhs=ones[0:1, 0:512], start=False, stop=True,
               skip_group_check=True)
            nc.vector.tensor_scalar(kf[hp][:, ts(c, 512)],
                                    p[:], 0.0, 1.0, MAX, MIN)

        def v_proj(m):
            """v n-tile m -> V pair tile m//2, slot m%2, clipped fp8."""
            p = pj.tile([128, 512], f32, tag="pj")
            for ck in range(CT):
                mm(p[:], xv_t[ck][:, ts(m, 128)], rhs=wv_t[ck][:],
                   start=(ck == 0), stop=False, skip_group_check=True)
            mm(p[:], ones[0:1, 0:128], rhs=tb[2], start=False, stop=True,
               skip_group_check=True)
            dst = Vt[m // 2][:, (m % 2) * 1024:(m % 2) * 1024 + 1024]
            dst = dst.rearrange("p (h x) -> p h x", x=128)[:, :, 0:64]
            nc.vector.tensor_scalar(dst, p[:].rearrange("p (h x) -> p h x", x=64),
                                    0.0, 1.0, MAX, MIN)

        def out_proj(m):
            """output n-tile m: contract all 4 oT d-tiles + bias, DMA out."""
            p = pj.tile([128, 512], f32, tag="pj")
            for hp in range(CT):
                mm(p[:], oT[hp][:, ts(m, 128)], rhs=wo_t[hp][:],
                   start=(hp == 0), stop=False, skip_group_check=True)
            mm(p[:], ones[0:1, 0:128], rhs=tb[3], start=False, stop=True,
               skip_group_check=True)
            y = yp.tile([128, 512], f32, tag="y")
            nc.vector.tensor_copy(y[:], p[:])
            nc.sync.dma_start(out[ts(m, 128), :], y[:])

        def sweep(ch, hp, interleave):
            """attention for q-chunk ch, head pair hp."""
            heads = (2 * hp, 2 * hp + 1)
            U = {h: up.tile([128, 512], f32, tag="U", name=f"U{h}_{ch}")
                 for h in heads}
            for a in range(4):
                for h in heads:
                    base = (h % 2) * 64
                    s = sc.tile([128, 1024], f32, tag="sc",
                                name=f"s{h}_{a}_{ch}")
                    for i in range(2):
                        m = 2 * a + i
                        lhs = kf[hp][base:base + 64, m * 128:(m + 2) * 128]
                        lhs = lhs.rearrange("p (s c) -> p s c", s=2)
                        rhs = qP[hp][base:base + 64, ch * 1024:(ch + 1) * 1024]
                        rhs = rhs.rearrange("p (s c) -> p s c", s=2)
                        mm(s[:, ts(i, 512)], lhs, rhs, start=True, stop=True,
                           perf_mode=DR, skip_group_check=True)
                    E = ep.tile([128, 1024], f8, tag="E", name=f"E{h}_{a}_{ch}")
                    nc.scalar.activation(E[:], s[:], Exp, bias=ebias[:],
                                         scale=0.125)
                    lhs = Vt[a][:].rearrange("p (s c) -> p s c", s=2)
                    lhs = lhs[:, :, h * 128:(h + 1) * 128]
                    mm(U[h][:], lhs, E[:].rearrange("p (s c) -> p s c", s=2),
                       start=(a == 0), stop=(a == 3), perf_mode=DR,
                       skip_group_check=True)
                interleave(a)
            for h in heads:
                rc = yp.tile([64, 512], f32, tag="rc")
                nc.vector.reciprocal(rc[:], U[h][64:128, :])
                base = (h % 2) * 64
                nc.vector.tensor_mul(oT[hp][base:base + 64, ts(ch, 512)],
                                     U[h][0:64, :], rc[:])

        def nop(a):
            pass

        # ---- phase order ----
        q_proj(0, 0)
        k_proj(0, 0)
        k_proj(0, 1)
        for m in range(NT):
            v_proj(m)

        def mk_interleave(ch, hp):
            # during sweep (ch, hp), project operands for the next sweep
            def f(a):
                if ch == 0:
                    if a == 0 and hp < 3:
                        q_proj(hp + 1, 0)
                    elif a == 1 and hp < 3:
                        k_proj(hp + 1, 0)
                    elif a == 2 and hp < 3:
                        k_proj(hp + 1, 1)
                    elif a == 1 and hp == 3:
                        q_proj(0, 1)
                        q_proj(1, 1)
                    elif a == 2 and hp == 3:
                        q_proj(2, 1)
                        q_proj(3, 1)
                else:
                    # out-projection of chunk-0 m-tiles rides under ch1 sweeps
                    if a == 1 and hp < 2:
                        out_proj(2 * hp)
                    elif a == 2 and hp < 2:
                        out_proj(2 * hp + 1)
            return f

        for ch in range(CH):
            for hp in range(CT):
                sweep(ch, hp, mk_interleave(ch, hp))
        for m in range(4, NT):
            out_proj(m)

    nc.compile()
    return nc


def get_nc():
    if "nc" not in _CACHE:
        _CACHE["nc"] = _build()
    return _CACHE["nc"]


def _prep_inputs(query, key, value, Wq, Wk, Wv, Wo, bn_params):
    """Host-side: shard + transpose + fold BN scale into weights."""
    query = np.ascontiguousarray(np.asarray(query, dtype=np.float32))
    key = np.ascontiguousarray(np.asarray(key, dtype=np.float32))
    value = np.ascontiguousarray(np.asarray(value, dtype=np.float32))
    bn = np.asarray(bn_params, dtype=np.float32)

    s = bn[:, 0] / np.sqrt(bn[:, 3] + EPS)      # [4, C]
    t = bn[:, 1] - bn[:, 2] * s                  # [4, C]

    def wprep(W, j):
        W = np.asarray(W, dtype=np.float32)
        return np.ascontiguousarray((W * s[j][:, None]).T)

    wqT, wkT, wvT, woT = (wprep(Wq, 0), wprep(Wk, 1), wprep(Wv, 2),
                          wprep(Wo, 3))
    tbias = np.ascontiguousarray(t)

    # [T, B, N, C] -> [8, C, N]
    def xT(x):
        return np.ascontiguousarray(
            x.reshape(N_CORES, N, C).transpose(0, 2, 1))

    qT, kT, vT = xT(query), xT(key), xT(value)

    in_maps = []
    for i in range(N_CORES):
        in_maps.append({
            "xq": qT[i], "xk": kT[i], "xv": vT[i],
            "wq": wqT, "wk": wkT, "wv": wvT, "wo": woT,
            "tbias": tbias,
        })
    return in_maps


def kernel(query, key, value, Wq, Wk, Wv, Wo, bn_params):
    from concourse.bass_utils import run_bass_kernel_spmd

    nc = get_nc()
    in_maps = _prep_inputs(query, key, value, Wq, Wk, Wv, Wo, bn_params)
    res = run_bass_kernel_spmd(nc, in_maps, core_ids=list(range(N_CORES)),
                               trace=False)
    T, B = 4, 2
    out = np.stack([res.results[i]["out"] for i in range(N_CORES)])
    return np.ascontiguousarray(out.reshape(T, B, N, C).astype(np.float32))


# revision 3
# speedup vs baseline: 1.3884x; 1.2297x over previous
"""Cross-attention kernel for Trainium2, SPMD over 8 NeuronCores.

Problem: T=4, B=2, NQ=NK=1024, C=512, H=8 heads (D=64).
  q = clip01(BN0(query @ Wq.T)); k = clip01(BN1(key @ Wk.T)); v = clip01(BN2(value @ Wv.T))
  per head: O = softmax(Q K^T / sqrt(D)) V
  out = BN3(concat(O) @ Wo.T)

Sharding: pure data-parallel, one (t, b) pair per core (T*B == 8 == n_cores).

v3 dataflow (fp8e4 + DoubleRow everywhere except the out-projection; the
ScalarE exp stream is the bottleneck the schedule is built around):
  - host ships query/key/value [C,N] and BN-folded Wq/Wk/Wv already in fp8e4
    (256KB each), Wo in f32r; BN biases stay f32 and are applied via K=1
    ones-row matmuls inside each PSUM accumulation group.
  - q/k/v projections: DoubleRow over c-tile pairs (2 mms of K=256 + bias mm
    per 512-col chunk); epilogues clip01 + emit fp8e4 operand tiles.
  - QK per (head, m): DoubleRow with k-slots (m, m+1) stationary and q-slots
    (data, zeros) moving -> S^T[m] at half PE cost; scores fill 2-bank PSUM
    tiles per m-pair.
  - exp: one ScalarE op per (head, m-pair) reads [128,1024] across 2 PSUM
    banks: E = exp(0.125*S - 2.75) -> fp8e4 (max ~190 < 240; the uniform
    shift cancels in softmax).
  - PV: genuine DoubleRow over m-pairs: lhsT = [V_h | ones64] pair tile
    [128,2,128], rhs = E pair [128,2,512]; U rows 0:64 = numerator, rows
    64:128 = denominator replicated via ones columns, so normalization is
    just DVE recip + mult (no partition broadcast).
  - out projection (f32r): chunk-0 m-tiles accumulate whole groups in PSUM
    under chunk-1 sweeps; chunk-1 m-tiles accumulate per-head-pair partials
    into an SBUF accumulator as each sweep's normalize completes, leaving
    only the hp3 partial + bias + final add on the tail.
"""

import numpy as np

import os

H, D, C, N = 8, 64, 512, 1024
CT = C // 128          # 4 c-tiles
NT = N // 128          # 8 n-tiles
CH = N // 512          # 2 free-dim chunks of 512
EPS = 1e-5
N_CORES = 8
K_WARM = int(os.environ.get("K_WARM", "0"))
EP_BUFS = int(os.environ.get("K_EP", "10"))
SC_BUFS = int(os.environ.get("K_SC", "2"))

_CACHE = {}


def _build():
    from contextlib import ExitStack

    import concourse.bass as bass
    import concourse.tile as tile
    from concourse import bacc, mybir

    f32 = mybir.dt.float32
    f32r = mybir.dt.float32r
    f8 = mybir.dt.float8e4
    DR = mybir.MatmulPerfMode.DoubleRow
    ts = bass.ts

    nc = bacc.Bacc("TRN2", target_bir_lowering=False, debug=False,
                   num_devices=N_CORES)

    xq = nc.dram_tensor("xq", [C, N], f8, kind="ExternalInput").ap()
    xk = nc.dram_tensor("xk", [C, N], f8, kind="ExternalInput").ap()
    xv = nc.dram_tensor("xv", [C, N], f8, kind="ExternalInput").ap()
    wq = nc.dram_tensor("wq", [C, C], f8, kind="ExternalInput").ap()
    wk = nc.dram_tensor("wk", [C, C], f8, kind="ExternalInput").ap()
    wv = nc.dram_tensor("wv", [C, C], f8, kind="ExternalInput").ap()
    wo = nc.dram_tensor("wo", [C, C], f32r, kind="ExternalInput").ap()
    tbias = nc.dram_tensor("tbias", [4, C], f32r, kind="ExternalInput").ap()
    out = nc.dram_tensor("out", [N, C], f32, kind="ExternalOutput").ap()

    Exp = mybir.ActivationFunctionType.Exp
    MAX, MIN = mybir.AluOpType.max, mybir.AluOpType.min
    ADD = mybir.AluOpType.add

    def mm(out_ap, lhsT, rhs, **kw):
        nc.tensor.matmul(out_ap, lhsT=lhsT, rhs=rhs, **kw)

    with tile.TileContext(nc) as tc, ExitStack() as ctx:
        sb = ctx.enter_context(tc.tile_pool(name="sb", bufs=1))
        ep = ctx.enter_context(tc.tile_pool(name="ep", bufs=EP_BUFS))
        yp = ctx.enter_context(tc.tile_pool(name="yp", bufs=3))
        sc = ctx.enter_context(tc.tile_pool(name="sc", bufs=SC_BUFS, space="PSUM"))
        up = ctx.enter_context(tc.tile_pool(name="up", bufs=2, space="PSUM"))
        pj = ctx.enter_context(tc.tile_pool(name="pj", bufs=2, space="PSUM"))

        ones = sb.tile([1, 512], f32r, tag="ones")
        nc.gpsimd.memset(ones[:].bitcast(f32), 1.0)
        ebias = sb.tile([128, 1], f32, tag="ebias")
        nc.gpsimd.memset(ebias[:], -2.75)
        # warm the ACT exp table while input DMAs stream
        junk = sb.tile([1, 8], f32, tag="junk")
        nc.scalar.activation(junk[:], ones[0:1, 0:8].bitcast(f32), Exp)
        for w in range(K_WARM):
            pw = pj.tile([128, 512], f32, tag="pj", name=f"warm{w}")
            mm(pw[:], ones[0:1, 0:128], rhs=ones[0:1, 0:512],
               start=True, stop=True, skip_group_check=True)

        # ---- input DMAs (DMA device is serial: order = need order) ----
        tbs = sb.tile([1, 4 * C], f32r, tag="tbs")
        nc.sync.dma_start(tbs[:].rearrange("p (j c) -> p j c", c=C),
                          tbias[:].rearrange("(j p) c -> p j c", p=1))
        tb = [tbs[0:1, ts(j, C)] for j in range(4)]

        def load_w8(name, ap, eng):
            # fp8 weights: [128, (ck, dout)]
            t = sb.tile([128, CT * C], f8, tag=name, name=name)
            eng.dma_start(t[:].rearrange("p (c d) -> p c d", d=C),
                          ap[:].rearrange("(c p) d -> p c d", p=128))
            return t

        def load_x8(name, ap, eng):
            # fp8 activations: [128, (ck, n)], loaded in column halves
            t = sb.tile([128, CT * N], f8, tag=name, name=name)
            tv = t[:].rearrange("p (c n) -> p c n", n=N)
            av = ap[:].rearrange("(c p) n -> p c n", p=128)
            def half(ch):
                eng.dma_start(tv[:, :, ts(ch, 512)], av[:, :, ts(ch, 512)])
            return t, half

        wq_t = load_w8("wq", wq, nc.sync)
        xq_t, xq_half = load_x8("xq", xq, nc.sync)
        xq_half(0)
        wk_t = load_w8("wk", wk, nc.scalar)
        xk_t, xk_half = load_x8("xk", xk, nc.scalar)
        xk_half(0)
        wv_t = load_w8("wv", wv, nc.sync)
        xv_t, xv_half = load_x8("xv", xv, nc.sync)
        xv_half(0)
        xq_half(1)
        xk_half(1)
        xv_half(1)
        wo_tt = sb.tile([128, CT * C], f32r, tag="wo", name="wo")
        nc.scalar.dma_start(wo_tt[:].rearrange("p (c d) -> p c d", d=C),
                            wo[:].rearrange("(c p) d -> p c d", p=128))
        wo_t = [wo_tt[:, ts(ck, C)] for ck in range(CT)]

        # ---- persistent fp8 operand tiles ----
        # qP[hp]: [128, (ch, slot, 512)]; slot1 = zeros (DoubleRow spacer)
        qP = [sb.tile([128, 2048], f8, tag=f"qP{j}", name=f"qP{j}")
              for j in range(CT)]
        # kf[hp]: [128, 1152]; m-major k columns + 128 pad cols
        kf = [sb.tile([128, 1152], f8, tag=f"kf{j}", name=f"kf{j}")
              for j in range(CT)]
        # V pair tiles: [128, (slot, head, 128)]; per head 64 V + 64 ones
        Vt = [sb.tile([128, 2048], f8, tag=f"V{a}", name=f"V{a}")
              for a in range(NT // 2)]
        for j in range(CT):
            z = qP[j][:].rearrange("p (c s n) -> p c s n", s=2, n=512)
            nc.gpsimd.memset(z[:, :, 1, :], 0.0)
            nc.gpsimd.memset(kf[j][:, 1024:1152], 0.0)
        for a in range(NT // 2):
            v4 = Vt[a][:].rearrange("p (s h x) -> p s h x", s=2, x=128)
            nc.gpsimd.memset(v4[:, :, :, 64:128], 1.0)

        # oT: normalized attention output [C, N] (lhsT of out-projection)
        oT = [sb.tile([128, N], f32r, tag=f"oT{j}", name=f"oT{j}")
              for j in range(CT)]
        # chunk-1 out-projection partial accumulator (m-tiles 4..7)
        y_acc = sb.tile([128, 4 * 512], f32, tag="y_acc")

        xq_v = xq_t[:].rearrange("p (c n) -> p c n", n=N)
        xk_v = xk_t[:].rearrange("p (c n) -> p c n", n=N)
        xv_v = xv_t[:].rearrange("p (c n) -> p c n", n=N)
        wq_v = wq_t[:].rearrange("p (c d) -> p c d", d=C)
        wk_v = wk_t[:].rearrange("p (c d) -> p c d", d=C)
        wv_v = wv_t[:].rearrange("p (c d) -> p c d", d=C)

        def qk_proj(w_v, x_v, bias_off, hp, c, dst):
            """transposed-orientation projection chunk (DR over c-pairs)."""
            p = pj.tile([128, 512], f32, tag="pj")
            for a in range(2):
                mm(p[:], w_v[:, 2 * a:2 * a + 2, ts(hp, 128)],
                   rhs=x_v[:, 2 * a:2 * a + 2, ts(c, 512)],
                   start=(a == 0), stop=False, perf_mode=DR,
                   skip_group_check=True)
            mm(p[:], tbs[0:1, bias_off + hp * 128:bias_off + hp * 128 + 128],
               rhs=ones[0:1, 0:512], start=False, stop=True,
               skip_group_check=True)
            nc.vector.tensor_scalar(dst, p[:], 0.0, 1.0, MAX, MIN)

        def q_proj(hp, ch):
            qk_proj(wq_v, xq_v, 0, hp, ch,
                    qP[hp][:, ch * 1024:ch * 1024 + 512])

        def k_proj(hp, c):
            qk_proj(wk_v, xk_v, C, hp, c, kf[hp][:, ts(c, 512)])

        def v_proj(m):
            """v n-tile m -> V pair tile m//2, slot m%2, clipped fp8."""
            p = pj.tile([128, 512], f32, tag="pj")
            for a in range(2):
                mm(p[:], xv_v[:, 2 * a:2 * a + 2, ts(m, 128)],
                   rhs=wv_v[:, 2 * a:2 * a + 2, :],
                   start=(a == 0), stop=False, perf_mode=DR,
                   skip_group_check=True)
            mm(p[:], ones[0:1, 0:128], rhs=tb[2], start=False, stop=True,
               skip_group_check=True)
            dst = Vt[m // 2][:, (m % 2) * 1024:(m % 2) * 1024 + 1024]
            dst = dst.rearrange("p (h x) -> p h x", x=128)[:, :, 0:64]
            nc.vector.tensor_scalar(dst, p[:].rearrange("p (h x) -> p h x", x=64),
                                    0.0, 1.0, MAX, MIN)

        def out_proj(m):
            """chunk-0 output n-tile: whole group in PSUM + bias, DMA out."""
            p = pj.tile([128, 512], f32, tag="pj")
            for hp in range(CT):
                mm(p[:], oT[hp][:, ts(m, 128)], rhs=wo_t[hp][:],
                   start=(hp == 0), stop=False, skip_group_check=True)
            mm(p[:], ones[0:1, 0:128], rhs=tb[3], start=False, stop=True,
               skip_group_check=True)
            y = yp.tile([128, 512], f32, tag="y")
            nc.vector.tensor_copy(y[:], p[:])
            nc.sync.dma_start(out[ts(m, 128), :], y[:])

        def out_part(hp, m):
            """chunk-1 partial: oT[hp] contribution for n-tile m (4..7)."""
            p = pj.tile([128, 512], f32, tag="pj")
            mm(p[:], oT[hp][:, ts(m, 128)], rhs=wo_t[hp][:],
               start=True, stop=(hp < 3), skip_group_check=True)
            if hp == 3:
                mm(p[:], ones[0:1, 0:128], rhs=tb[3], start=False, stop=True,
                   skip_group_check=True)
            acc = y_acc[:, ts(m - 4, 512)]
            if hp == 0:
                nc.vector.tensor_copy(acc, p[:])
            elif hp < 3:
                nc.vector.tensor_tensor(acc, p[:], acc, ADD)
            else:
                y = yp.tile([128, 512], f32, tag="y")
                nc.vector.tensor_tensor(y[:], p[:], acc, ADD)
                nc.sync.dma_start(out[ts(m, 128), :], y[:])

        def sweep(ch, hp, interleave, defer_pv=False):
            """attention for q-chunk ch, head pair hp."""
            heads = (2 * hp, 2 * hp + 1)
            U = {h: up.tile([128, 512], f32, tag="U", name=f"U{h}_{ch}")
                 for h in heads}
            pv = []
            for a in range(4):
                for h in heads:
                    base = (h % 2) * 64
                    s = sc.tile([128, 1024], f32, tag="sc",
                                name=f"s{h}_{a}_{ch}")
                    for i in range(2):
                        m = 2 * a + i
                        lhs = kf[hp][base:base + 64, m * 128:(m + 2) * 128]
                        lhs = lhs.rearrange("p (s c) -> p s c", s=2)
                        rhs = qP[hp][base:base + 64, ch * 1024:(ch + 1) * 1024]
                        rhs = rhs.rearrange("p (s c) -> p s c", s=2)
                        mm(s[:, ts(i, 512)], lhs, rhs, start=True, stop=True,
                           perf_mode=DR, skip_group_check=True)
                    E = ep.tile([128, 1024], f8, tag="E", name=f"E{h}_{a}_{ch}")
                    nc.scalar.activation(E[:], s[:], Exp, bias=ebias[:],
                                         scale=0.125)
                    if defer_pv:
                        pv.append((h, a, E))
                    else:
                        pv_mm(U, h, a, E)
                interleave(a)
            for h, a, E in pv:
                pv_mm(U, h, a, E)
            for h in heads:
                rc = yp.tile([64, 512], f32, tag="rc")
                nc.vector.reciprocal(rc[:], U[h][64:128, :])
                base = (h % 2) * 64
                nc.vector.tensor_mul(oT[hp][base:base + 64, ts(ch, 512)],
                                     U[h][0:64, :], rc[:])

        def pv_mm(U, h, a, E):
            lhs = Vt[a][:].rearrange("p (s c) -> p s c", s=2)
            lhs = lhs[:, :, h * 128:(h + 1) * 128]
            mm(U[h][:], lhs, E[:].rearrange("p (s c) -> p s c", s=2),
               start=(a == 0), stop=(a == 3), perf_mode=DR,
               skip_group_check=True)

        # ---- phase order ----
        q_proj(0, 0)
        k_proj(0, 0)
        k_proj(0, 1)
        for m in range(4):
            v_proj(m)

        def mk_interleave(ch, hp):
            # during sweep (ch, hp), prepare operands for later sweeps
            def f(a):
                if ch == 0 and hp == 0:
                    if a == 0:
                        v_proj(4)
                        v_proj(5)
                    elif a == 1:
                        v_proj(6)
                        v_proj(7)
                    elif a == 2:
                        q_proj(1, 0)
                        k_proj(1, 0)
                    elif a == 3:
                        k_proj(1, 1)
                elif ch == 0:
                    if a == 0 and hp < 3:
                        q_proj(hp + 1, 0)
                    elif a == 1 and hp < 3:
                        k_proj(hp + 1, 0)
                    elif a == 2 and hp < 3:
                        k_proj(hp + 1, 1)
                    elif a == 1 and hp == 3:
                        q_proj(0, 1)
                        q_proj(1, 1)
                    elif a == 2 and hp == 3:
                        q_proj(2, 1)
                        q_proj(3, 1)
                else:
                    # chunk-0 whole out-projections + chunk-1 partials
                    if a == 0 and hp > 0:
                        out_part(hp - 1, 4)
                        out_part(hp - 1, 5)
                    elif a == 1:
                        if hp < 2:
                            out_proj(2 * hp)
                        if hp > 0:
                            out_part(hp - 1, 6)
                    elif a == 2:
                        if hp < 2:
                            out_proj(2 * hp + 1)
                        if hp > 0:
                            out_part(hp - 1, 7)
            return f

        for ch in range(CH):
            for hp in range(CT):
                sweep(ch, hp, mk_interleave(ch, hp),
                      defer_pv=(ch == 0 and hp == 0))
        for m in range(4, NT):
            out_part(3, m)

    nc.compile()
    return nc


def get_nc():
    if "nc" not in _CACHE:
        _CACHE["nc"] = _build()
    return _CACHE["nc"]


def _prep_inputs(query, key, value, Wq, Wk, Wv, Wo, bn_params):
    """Host-side: shard + transpose + fold BN scale into weights + fp8 cast."""
    import ml_dtypes
    f8 = ml_dtypes.float8_e4m3

    query = np.ascontiguousarray(np.asarray(query, dtype=np.float32))
    key = np.ascontiguousarray(np.asarray(key, dtype=np.float32))
    value = np.ascontiguousarray(np.asarray(value, dtype=np.float32))
    bn = np.asarray(bn_params, dtype=np.float32)

    s = bn[:, 0] / np.sqrt(bn[:, 3] + EPS)      # [4, C]
    t = bn[:, 1] - bn[:, 2] * s                  # [4, C]

    def wprep(W, j, dt):
        W = np.asarray(W, dtype=np.float32)
        return np.ascontiguousarray((W * s[j][:, None]).T.astype(dt))

    wqT, wkT, wvT = (wprep(Wq, 0, f8), wprep(Wk, 1, f8), wprep(Wv, 2, f8))
    woT = wprep(Wo, 3, np.float32)
    tbias = np.ascontiguousarray(t)

    # [T, B, N, C] -> [8, C, N] in fp8
    def xT(x):
        return np.ascontiguousarray(
            x.reshape(N_CORES, N, C).transpose(0, 2, 1).astype(f8))

    qT, kT, vT = xT(query), xT(key), xT(value)

    in_maps = []
    for i in range(N_CORES):
        in_maps.append({
            "xq": qT[i], "xk": kT[i], "xv": vT[i],
            "wq": wqT, "wk": wkT, "wv": wvT, "wo": woT,
            "tbias": tbias,
        })
    return in_maps


def kernel(query, key, value, Wq, Wk, Wv, Wo, bn_params):
    from concourse.bass_utils import run_bass_kernel_spmd

    nc = get_nc()
    in_maps = _prep_inputs(query, key, value, Wq, Wk, Wv, Wo, bn_params)
    res = run_bass_kernel_spmd(nc, in_maps, core_ids=list(range(N_CORES)),
                               trace=False)
    T, B = 4, 2
    out = np.stack([res.results[i]["out"] for i in range(N_CORES)])
    return np.ascontiguousarray(out.reshape(T, B, N, C).astype(np.float32))
